# revision 27
# baseline (speedup 1.0000x reference)
"""Trainium2 Bass kernel for nn_EquivariantMultiheadAttention.

Sharding: query-point axis (dim 1) split across 8 cores (16 points each).

Key restructure vs the straightforward version: the ky-MLP depends on just
two scalars (f_key, f_query), so exp(silu(ky3(f_q, f_k))) is approximated by
2D Chebyshev interpolation (32 nodes/dim, max rel err ~6e-6).  That yields a
rank-32-per-channel factorization E_y = U @ Psi^T evaluated as one K=128
matmul per group-pair on the tensor engine — eliminating half of all SILU
work from the activation engine (the kernel bottleneck).  mask*f_k / mask
are folded into the Psi factors so the softmax numerator/denominator come
out of two matmuls + DVE mul/reduce against exp(silu(kg3)).

Device phase 1 per tile (b, q, sq) of 512 keys: kg L1 matmul (K=9) -> SiLU
-> kg L2 as 4 concurrent 32x32 tile-positioned matmuls (output bands rotated
by tile%4 so 4 consecutive tiles pack the full PE array) -> SiLU -> L3
(zero-padded M=32 matmuls accumulating 32 tiles into one dense PSUM bank).
Phase 2 (Exp table): per group-pair (32 tiles): 2 matmuls (num/den factor
maps), exp, DVE mul/reduce/normalize, residual + query mask.
Final w_out projection happens host-side on the tiny [B,N,S,4] result.
"""
import numpy as np
import ml_dtypes

BF16 = ml_dtypes.bfloat16

B, N, S, DG, C, HID, COUT = 2, 128, 4, 8, 4, 32, 8
NCORE = 8
QL = N // NCORE          # 16 query points per core
KEY = N * S              # 512 keys
T = B * QL * S           # 128 tiles per core
M = 32                   # Chebyshev nodes per dim (= rank per channel)
NPAIR = 4                # group-pairs per core (32 tiles each)

_PROG = None


def _cheb_fit(inp):
    """Per (b, c): Chebyshev nodes + grid values G of exp(silu(ky3))."""
    cf = np.asarray(inp["coset_functions"], np.float64)
    kyW1 = np.asarray(inp["ky_W1"], np.float64)
    kyb1 = np.asarray(inp["ky_b1"], np.float64)
    kyW2 = np.asarray(inp["ky_W2"], np.float64)
    kyb2 = np.asarray(inp["ky_b2"], np.float64)
    kyW3 = np.asarray(inp["ky_W3"], np.float64)
    kyb3 = np.asarray(inp["ky_b3"], np.float64)

    def silu(x):
        return x / (1.0 + np.exp(-x))

    j = np.arange(M)
    cw = (-1.0) ** j * np.sin((2 * j + 1) * np.pi / (2 * M))
    nodes = np.zeros((B, C, M))
    G = np.zeros((B, C, M, M))
    for b in range(B):
        for c in range(C):
            f = cf[b, :, :, c].ravel()
            lo, hi = f.min(), f.max()
            x = (lo + hi) / 2 + (hi - lo) / 2 * np.cos((2 * j + 1) * np.pi / (2 * M))
            nodes[b, c] = x
            FK, FQ = np.meshgrid(x, x, indexing="ij")
            h = silu(FK[..., None] * kyW1[c, :, 0] + FQ[..., None] * kyW1[c, :, 1] + kyb1[c])
            h = silu(h @ kyW2[c].T + kyb2[c])
            G[b, c] = np.exp(silu(h @ kyW3[c].T + kyb3[c])[..., 0])  # [fk_node, fq_node]
    return nodes, G, cw


def _lag(x, nd, cw):
    """Barycentric Lagrange basis values: [len(x), M]."""
    d = x[:, None] - nd[None, :]
    ex = np.isclose(d, 0.0, atol=1e-12)
    d = np.where(ex, 1.0, d)
    L = cw[None, :] / d
    L = L / L.sum(1, keepdims=True)
    L = np.where(ex.any(1)[:, None], ex.astype(np.float64), L)
    return L


def _pack_globals(inp):
    cf = np.asarray(inp["coset_functions"], np.float64)
    mask = np.asarray(inp["mask"]).astype(np.float64)
    kgW1 = np.asarray(inp["kg_W1"], np.float32)
    kgW2 = np.asarray(inp["kg_W2"], np.float32)
    kgW3 = np.asarray(inp["kg_W3"], np.float32)
    out = {}
    # kg L1: lhsT [9, 128] (8 g dims + bias row)
    w1g = np.zeros((DG + 1, 128), np.float32)
    for c in range(C):
        w1g[0:DG, c * 32:(c + 1) * 32] = kgW1[c].T
    w1g[DG, :] = np.asarray(inp["kg_b1"], np.float32).reshape(128)
    out["w1g"] = w1g.astype(BF16)
    # kg L2 blocks: band c = W2[c].T  [128, 32]
    w2rep = np.zeros((128, 32), np.float32)
    for c in range(C):
        w2rep[c * 32:(c + 1) * 32, :] = kgW2[c].T
    out["w2rep"] = w2rep.astype(BF16)
    # kg L3, 4 rotation variants: [128, 4*256]
    w3grot = np.zeros((128, 4 * 256), np.float32)
    for rho in range(4):
        for s in range(8):
            for c in range(C):
                cc = (c + rho) % 4
                w3grot[cc * 32:(cc + 1) * 32, 256 * rho + 36 * s + c] = kgW3[c, 0, :]
    out["w3grot"] = w3grot.astype(BF16)
    # biases: cols 0-3 = b2 rotated; col 4 = b3 pattern
    bias128 = np.zeros((128, 8), np.float32)
    b2 = np.asarray(inp["kg_b2"], np.float32)
    for rho in range(4):
        for c in range(C):
            cc = (c + rho) % 4
            bias128[cc * 32:(cc + 1) * 32, rho] = b2[c]
    bias128[:, 4] = np.tile(np.asarray(inp["kg_b3"], np.float32).reshape(C), 32)
    out["bias128"] = bias128
    # Psi factors (num/den) per batch: [128, 2*KEY]
    nodes, G, cw = _cheb_fit(inp)
    psin = np.zeros((128, B * KEY), np.float64)
    psid = np.zeros((128, B * KEY), np.float64)
    for b in range(B):
        mk = mask[b].ravel()
        for c in range(C):
            fk = cf[b, :, :, c].ravel()
            Lk = _lag(fk, nodes[b, c], cw)            # [KEY, M]
            psin[32 * c:32 * c + 32, b * KEY:(b + 1) * KEY] = (Lk * (mk * fk)[:, None]).T
            psid[32 * c:32 * c + 32, b * KEY:(b + 1) * KEY] = (Lk * mk[:, None]).T
    out["psin2"] = psin.astype(BF16)
    out["psid2"] = psid.astype(BF16)
    return out, (nodes, G, cw)


def _pack_core(core, inp, aux):
    nodes, G, cw = aux
    g = np.asarray(inp["pairwise_g"], np.float32)
    cf = np.asarray(inp["coset_functions"], np.float64)
    mask = np.asarray(inp["mask"]).astype(np.float32)
    qs = slice(core * QL, (core + 1) * QL)
    out = {}
    # g tiles, pair-ordered: pair p = tiles (t, t+4), t = 8*(p//4) + p%4
    gt = g[:, qs]                                        # [B,QL,N,S,S,DG]
    g_t = np.zeros((T, DG + 1, KEY), np.float32)
    g_t[:, 0:DG, :] = gt.transpose(0, 1, 3, 5, 2, 4).reshape(T, DG, KEY)
    g_t[:, DG, :] = 1.0
    p_arr = np.arange(64)
    tA = 8 * (p_arr // 4) + (p_arr % 4)
    g_t2 = np.concatenate([g_t[tA], g_t[tA + 4]], axis=2)  # [64, 9, 1024]
    out["g_t2"] = np.ascontiguousarray(g_t2.astype(BF16))
    # Upack + residual/mask smalls
    upack = np.zeros((128, 128 * NPAIR), np.float64)
    small = np.zeros((128, 8), np.float32)
    cfq = cf[:, qs]                                      # [B,QL,S,C]
    for t in range(T):
        b, r = divmod(t, QL * S)
        ql, sq = divmod(r, S)
        P, u = divmod(t, 32)
        cg, s = u % 4, u // 4
        for c in range(C):
            fq = cfq[b, ql, sq, c]
            u_vec = G[b, c] @ _lag(np.array([fq]), nodes[b, c], cw)[0]
            row = 32 * cg + 4 * s + c
            upack[32 * c:32 * c + 32, 128 * P + row] = u_vec
            small[row, P] = fq
            small[row, 4 + P] = mask[b, core * QL + ql, sq]
    out["upack"] = upack.astype(BF16)
    out["small128"] = small
    return out


def _build_program():
    from contextlib import ExitStack
    import concourse.bass as bass
    import concourse.tile as tile
    import concourse.mybir as mybir
    from concourse import bacc
    import bass_rust

    f32 = mybir.dt.float32
    bf16 = mybir.dt.bfloat16
    AF = mybir.ActivationFunctionType
    ALU = mybir.AluOpType

    nc = bacc.Bacc("TRN2", target_bir_lowering=False, debug=False,
                   enable_asserts=False, num_devices=NCORE)

    din = {}
    for name, shape, dt in (
        ("g_t2", [64, DG + 1, 2 * KEY], bf16),
        ("w1g", [DG + 1, 128], bf16),
        ("w2rep", [128, 32], bf16),
        ("w3grot", [128, 4 * 256], bf16),
        ("bias128", [128, 8], f32),
        ("upack", [128, 128 * NPAIR], bf16),
        ("psin2", [128, B * KEY], bf16),
        ("psid2", [128, B * KEY], bf16),
        ("small128", [128, 8], f32),
    ):
        din[name] = nc.dram_tensor(name, shape, dt, kind="ExternalInput").ap()
    dout = nc.dram_tensor("out128", [128, NPAIR], f32, kind="ExternalOutput").ap()

    with tile.TileContext(nc) as tc, ExitStack() as ctx:
        const = ctx.enter_context(tc.tile_pool(name="const", bufs=1))
        gp = ctx.enter_context(tc.tile_pool(name="gp", bufs=3))
        work = ctx.enter_context(tc.tile_pool(name="work", bufs=2))
        ps = ctx.enter_context(tc.tile_pool(name="ps", bufs=1, space="PSUM"))
        ep = ctx.enter_context(tc.tile_pool(name="ep", bufs=2))

        # --- constants to SBUF (upack first: the warm-up burst needs it) ---
        upack_s = const.tile([128, 128 * NPAIR], bf16, name="upack_s")
        nc.sync.dma_start(upack_s[:], din["upack"][:])
        w1g_s = const.tile([DG + 1, 128], bf16, name="w1g_s")
        nc.sync.dma_start(w1g_s[:], din["w1g"][:])
        w2rep_s = const.tile([128, 32], bf16, name="w2rep_s")
        nc.sync.dma_start(w2rep_s[:], din["w2rep"][:])
        bias128_s = const.tile([128, 8], f32, name="bias128_s")
        nc.sync.dma_start(bias128_s[:], din["bias128"][:])
        small128_s = const.tile([128, 8], f32, name="small128_s")
        nc.sync.dma_start(small128_s[:], din["small128"][:])
        # big consts DMA'd from inside the loop (after the first g tiles)
        w3grot_s = const.tile([128, 4 * 256], bf16, name="w3grot_s")
        psin2_s = const.tile([128, B * KEY], bf16, name="psin2_s")
        psid2_s = const.tile([128, B * KEY], bf16, name="psid2_s")
        logits_all = const.tile([128, NPAIR * KEY], f32, name="logits_all")
        pfac_s = const.tile([128, NPAIR * 2 * KEY], f32, name="pfac_s")
        out_s = const.tile([128, NPAIR], f32, name="out_s")

        # --- HAM warm-up: 15 dense FULL-ARRAY (K=128, M=128) matmuls.  The
        # clock gate only *latches* 8/8 under high array occupancy, but any
        # activity then *keeps* it warm — so one full burst up front makes
        # the whole tile-packed pipeline run at 2.4 GHz. ---
        scratch = ps.tile([128, KEY], f32, tag="warm", bufs=1, name="scratch")
        for _ in range(9):
            nc.tensor.matmul(scratch[:], upack_s[:, 0:128], upack_s[:, 0:KEY],
                             start=True, stop=True)

        def warm_fill(n):
            # Dependency-free full-array (K=128) short matmuls: the PE runs
            # them whenever it would otherwise idle, so the HAM activity
            # monitor never sees an idle window and 2.4 GHz persists.
            for _ in range(n):
                nc.tensor.matmul(scratch[:, 0:128], upack_s[:, 0:128],
                                 upack_s[:, 0:128], start=True, stop=True)

        # --- E_y factor maps: 8 matmuls + DVE copies to SBUF, emitted one
        # per early pipeline step (own 1-bank psum tag; fills PE gaps). ---
        def fac_stage(k):
            P, half = divmod(k, 2)
            b = P // (NPAIR // B)
            src = psin2_s if half == 0 else psid2_s
            pp = ps.tile([128, KEY], f32, tag="fac", bufs=1, name="ppF")
            nc.tensor.matmul(pp[:], upack_s[:, 128 * P:128 * (P + 1)],
                             src[:, b * KEY:(b + 1) * KEY],
                             start=True, stop=True, tile_position=(0, 0))
            nc.vector.tensor_copy(
                pfac_s[:, (2 * P + half) * KEY:(2 * P + half + 1) * KEY], pp[:])

        gts = {}
        ps1s = {}
        h1s = {}
        ps2s = {}
        h2s = {}
        ps3s = {}
        state = {"last": None}

        def dma_stage(p):
            gt = gp.tile([DG + 1, 2 * KEY], bf16, tag="gt", name="gt")
            nc.sync.dma_start(gt[:], din["g_t2"][p])
            gts[p] = gt

        def l1_stage(p):
            gt = gts.pop(p)
            pA = ps.tile([128, 2 * KEY], f32, tag="psL1", bufs=1, name="pA")
            nc.tensor.matmul(pA[:, 0:KEY], w1g_s[:], gt[:, 0:KEY],
                             start=True, stop=True, tile_position=(0, 0))
            nc.tensor.matmul(pA[:, KEY:2 * KEY], w1g_s[:], gt[:, KEY:2 * KEY],
                             start=True, stop=True, tile_position=(0, 0))
            ps1s[p] = pA

        def s1_stage(p):
            pA = ps1s.pop(p)
            h1 = work.tile([128, 2 * KEY], bf16, tag="h1", bufs=2, name="h1")
            nc.scalar.activation(h1[:], pA[:], AF.Silu, bias=0.0)
            h1s[p] = h1

        def l2_stage(p):
            rho = p % 4
            h1 = h1s.pop(p)
            pB = ps.tile([128, 2 * KEY], f32, tag="psH2", bufs=1, name="pB")
            for half in range(2):
                for c in range(C):
                    cc = (c + rho) % 4
                    nc.tensor.matmul(
                        pB[32 * cc:32 * cc + 32, half * KEY:(half + 1) * KEY],
                        w2rep_s[32 * c:32 * c + 32, :],
                        h1[32 * c:32 * c + 32, half * KEY:(half + 1) * KEY],
                        start=True, stop=True, tile_position=(32 * c, 32 * cc))
            ps2s[p] = pB

        def s2_stage(p):
            rho = p % 4
            pB = ps2s.pop(p)
            h2 = work.tile([128, 2 * KEY], bf16, tag="h2", bufs=2, name="h2")
            nc.scalar.activation(h2[:], pB[:], AF.Silu,
                                 bias=bias128_s[:, rho:rho + 1])
            h2s[p] = h2

        def l3_stage(p):
            rho = p % 4
            chunk = p // 4
            P = chunk // 4
            s0 = 2 * (chunk % 4)
            if p % 16 == 0:
                ps3s[P] = ps.tile([128, KEY], f32, tag="psL3", bufs=2, name="ps3")
            ps3 = ps3s[P]
            h2 = h2s.pop(p)
            for half in range(2):
                s = s0 + half
                nc.tensor.matmul(
                    ps3[32 * rho:32 * rho + 32, :],
                    w3grot_s[:, 256 * rho + 32 * s:256 * rho + 32 * s + 32],
                    h2[:, half * KEY:(half + 1) * KEY],
                    start=(s == 0), stop=(s == 7), tile_position=(0, 32 * rho))
            if p % 16 == 15:
                ps3s.pop(P)
                h = nc.scalar.activation(
                    logits_all[:, P * KEY:(P + 1) * KEY], ps3[:], AF.Silu,
                    bias=bias128_s[:, 4:5])
                state["last"] = h.ins

        # ============ phase 1: 5-deep software pipeline over 64 pairs ======
        # silu1 leads silu2 by one pair so the L2 matmuls of pair p hide
        # under silu1(p+1) instead of stalling the ACT engine.
        for step in range(64 + 4):
            if step < 64:
                dma_stage(step)
            if 3 <= step <= 66:
                l2_stage(step - 3)
            if 1 <= step <= 64:
                l1_stage(step - 1)
            if 2 <= step <= 65:
                s1_stage(step - 2)
            if 3 <= step <= 66:
                s2_stage(step - 3)
            if step >= 4:
                l3_stage(step - 4)
            if step == 1:
                nc.sync.dma_start(w3grot_s[:], din["w3grot"][:])
            if step == 2:
                nc.sync.dma_start(psin2_s[:], din["psin2"][:])
            if step == 3:
                nc.sync.dma_start(psid2_s[:], din["psid2"][:])
            if 10 <= step <= 17:
                fac_stage(step - 10)
            warm_fill(2)
        last_silu = state["last"]

        # ============ phase 2: exp + aggregate against factor maps =========
        for P in range(NPAIR):
            e = ep.tile([128, KEY], f32, tag="e", name="e")
            h = nc.scalar.activation(e[:], logits_all[:, P * KEY:(P + 1) * KEY],
                                     AF.Exp)
            bass_rust.add_dep_helper(h.ins, last_silu,
                                     reason="act-table phase barrier")
            import os as _os
            use_fused = _os.environ.get("K_FUSED_RED", "1") == "1"
            scrN = ep.tile([128, KEY], f32, tag="scrN", name="scrN")
            num = ep.tile([128, 1], f32, tag="num", name="num")
            scrD = ep.tile([128, KEY], f32, tag="scrD", name="scrD")
            den = ep.tile([128, 1], f32, tag="den", name="den")
            pn = pfac_s[:, P * 2 * KEY:P * 2 * KEY + KEY]
            pd = pfac_s[:, P * 2 * KEY + KEY:(P + 1) * 2 * KEY]
            if use_fused:
                nc.vector.affine_mul_reduce(scrN[:], num[:], e[:], pn, 1.0, 0.0)
                nc.vector.affine_mul_reduce(scrD[:], den[:], e[:], pd, 1.0, 0.0)
            else:
                nc.vector.tensor_mul(scrN[:], e[:], pn)
                nc.vector.tensor_reduce(num[:], scrN[:], mybir.AxisListType.X, ALU.add)
                nc.vector.tensor_mul(scrD[:], e[:], pd)
                nc.vector.tensor_reduce(den[:], scrD[:], mybir.AxisListType.X, ALU.add)
            rden = ep.tile([128, 1], f32, tag="rden", name="rden")
            nc.vector.reciprocal(rden[:], den[:])
            agg = ep.tile([128, 1], f32, tag="agg", name="agg")
            nc.vector.tensor_mul(agg[:], num[:], rden[:])
            res = ep.tile([128, 1], f32, tag="res", name="res")
            nc.vector.tensor_add(res[:], agg[:], small128_s[:, P:P + 1])
            nc.vector.tensor_mul(out_s[:, P:P + 1], res[:],
                                 small128_s[:, 4 + P:5 + P])
        nc.sync.dma_start(dout[:], out_s[:])

    nc.compile()
    return nc


def _get_program():
    global _PROG
    if _PROG is None:
        _PROG = _build_program()
    return _PROG


def _build_inmaps(inp):
    gl, aux = _pack_globals(inp)
    in_maps = []
    for core in range(NCORE):
        m = dict(gl)
        m.update(_pack_core(core, inp, aux))
        in_maps.append({k: np.ascontiguousarray(v) for k, v in m.items()})
    return in_maps


def kernel(**inputs) -> np.ndarray:
    from concourse.bass_utils import run_bass_kernel_spmd

    inp = {k: np.asarray(v) for k, v in inputs.items()}
    w_out = np.asarray(inp["w_out"], np.float32)
    in_maps = _build_inmaps(inp)
    nc = _get_program()
    res = run_bass_kernel_spmd(nc, in_maps, core_ids=list(range(NCORE)))

    cf_out = np.zeros((B, N, S, C), np.float32)
    for core in range(NCORE):
        OUT = res.results[core]["out128"]                  # [128, NPAIR]
        # row = 32*cg + 4*s + c, col = P;  t = 32*P + 4*s + cg
        arr = OUT.reshape(4, 8, C, NPAIR).transpose(3, 1, 0, 2).reshape(T, C)
        arr = arr.reshape(B, QL, S, C)
        cf_out[:, core * QL:(core + 1) * QL] = arr
    return (cf_out @ w_out.T).astype(np.float32)


# revision 28
# speedup vs baseline: 1.2235x; 1.2235x over previous
"""Trainium2 Bass kernel for nn_EquivariantMultiheadAttention.

Sharding: query-point axis (dim 1) split across 8 cores (16 points each).

Key restructure vs the straightforward version: the ky-MLP depends on just
two scalars (f_key, f_query), so exp(silu(ky3(f_q, f_k))) is approximated by
2D Chebyshev interpolation (32 nodes/dim, max rel err ~6e-6).  That yields a
rank-32-per-channel factorization E_y = U @ Psi^T evaluated as one K=128
matmul per group-pair on the tensor engine — eliminating half of all SILU
work from the activation engine (the kernel bottleneck).  mask*f_k / mask
are folded into the Psi factors so the softmax numerator/denominator come
out of two matmuls + DVE mul/reduce against exp(silu(kg3)).

Device phase 1 per tile (b, q, sq) of 512 keys: kg L1 matmul (K=9) -> SiLU
-> kg L2 as 4 concurrent 32x32 tile-positioned matmuls (output bands rotated
by tile%4 so 4 consecutive tiles pack the full PE array) -> SiLU -> L3
(zero-padded M=32 matmuls accumulating 32 tiles into one dense PSUM bank).
Phase 2 (Exp table): per group-pair (32 tiles): 2 matmuls (num/den factor
maps), exp, DVE mul/reduce/normalize, residual + query mask.
Final w_out projection happens host-side on the tiny [B,N,S,4] result.
"""
import numpy as np
import ml_dtypes

BF16 = ml_dtypes.bfloat16

B, N, S, DG, C, HID, COUT = 2, 128, 4, 8, 4, 32, 8
NCORE = 8
QL = N // NCORE          # 16 query points per core
KEY = N * S              # 512 keys
T = B * QL * S           # 128 tiles per core
M = 32                   # Chebyshev nodes per dim (= rank per channel)
NPAIR = 4                # group-pairs per core (32 tiles each)

_PROG = None


def _cheb_fit(inp):
    """Per (b, c): Chebyshev nodes + grid values G of exp(silu(ky3))."""
    cf = np.asarray(inp["coset_functions"], np.float64)
    kyW1 = np.asarray(inp["ky_W1"], np.float64)
    kyb1 = np.asarray(inp["ky_b1"], np.float64)
    kyW2 = np.asarray(inp["ky_W2"], np.float64)
    kyb2 = np.asarray(inp["ky_b2"], np.float64)
    kyW3 = np.asarray(inp["ky_W3"], np.float64)
    kyb3 = np.asarray(inp["ky_b3"], np.float64)

    def silu(x):
        return x / (1.0 + np.exp(-x))

    j = np.arange(M)
    cw = (-1.0) ** j * np.sin((2 * j + 1) * np.pi / (2 * M))
    nodes = np.zeros((B, C, M))
    G = np.zeros((B, C, M, M))
    for b in range(B):
        for c in range(C):
            f = cf[b, :, :, c].ravel()
            lo, hi = f.min(), f.max()
            x = (lo + hi) / 2 + (hi - lo) / 2 * np.cos((2 * j + 1) * np.pi / (2 * M))
            nodes[b, c] = x
            FK, FQ = np.meshgrid(x, x, indexing="ij")
            h = silu(FK[..., None] * kyW1[c, :, 0] + FQ[..., None] * kyW1[c, :, 1] + kyb1[c])
            h = silu(h @ kyW2[c].T + kyb2[c])
            G[b, c] = np.exp(silu(h @ kyW3[c].T + kyb3[c])[..., 0])  # [fk_node, fq_node]
    return nodes, G, cw


def _lag(x, nd, cw):
    """Barycentric Lagrange basis values: [len(x), M]."""
    d = x[:, None] - nd[None, :]
    ex = np.isclose(d, 0.0, atol=1e-12)
    d = np.where(ex, 1.0, d)
    L = cw[None, :] / d
    L = L / L.sum(1, keepdims=True)
    L = np.where(ex.any(1)[:, None], ex.astype(np.float64), L)
    return L


def _pack_globals(inp):
    cf = np.asarray(inp["coset_functions"], np.float64)
    mask = np.asarray(inp["mask"]).astype(np.float64)
    kgW1 = np.asarray(inp["kg_W1"], np.float32)
    kgW2 = np.asarray(inp["kg_W2"], np.float32)
    kgW3 = np.asarray(inp["kg_W3"], np.float32)
    out = {}
    # kg L1: lhsT [128, 128], rows 0-7 = g weights, row 8 = bias, rest 0
    # (K padded to 128 so the matmul streams through the full PE array).
    w1g = np.zeros((128, 128), np.float32)
    for c in range(C):
        w1g[0:DG, c * 32:(c + 1) * 32] = kgW1[c].T
    w1g[DG, :] = np.asarray(inp["kg_b1"], np.float32).reshape(128)
    out["w1g"] = w1g.astype(BF16)
    # kg L2: dense block-diagonal [128, 128] (full-array stream)
    w2full = np.zeros((128, 128), np.float32)
    for c in range(C):
        w2full[c * 32:(c + 1) * 32, c * 32:(c + 1) * 32] = kgW2[c].T
    out["w2full"] = w2full.astype(BF16)
    # kg L3: [128, 256], col 36s+c holds W3g[c] (s-slot packing)
    w3g = np.zeros((128, 256), np.float32)
    for s in range(8):
        for c in range(C):
            w3g[c * 32:(c + 1) * 32, 36 * s + c] = kgW3[c, 0, :]
    out["w3g"] = w3g.astype(BF16)
    # biases: col 0 = b2; col 4 = b3 pattern
    bias128 = np.zeros((128, 8), np.float32)
    bias128[:, 0] = np.asarray(inp["kg_b2"], np.float32).reshape(128)
    bias128[:, 4] = np.tile(np.asarray(inp["kg_b3"], np.float32).reshape(C), 32)
    out["bias128"] = bias128
    # Psi factors (num/den) per batch: [128, 2*KEY]
    nodes, G, cw = _cheb_fit(inp)
    psin = np.zeros((128, B * KEY), np.float64)
    psid = np.zeros((128, B * KEY), np.float64)
    for b in range(B):
        mk = mask[b].ravel()
        for c in range(C):
            fk = cf[b, :, :, c].ravel()
            Lk = _lag(fk, nodes[b, c], cw)            # [KEY, M]
            psin[32 * c:32 * c + 32, b * KEY:(b + 1) * KEY] = (Lk * (mk * fk)[:, None]).T
            psid[32 * c:32 * c + 32, b * KEY:(b + 1) * KEY] = (Lk * mk[:, None]).T
    out["psin2"] = psin.astype(BF16)
    out["psid2"] = psid.astype(BF16)
    return out, (nodes, G, cw)


def _pack_core(core, inp, aux):
    nodes, G, cw = aux
    g = np.asarray(inp["pairwise_g"], np.float32)
    cf = np.asarray(inp["coset_functions"], np.float64)
    mask = np.asarray(inp["mask"]).astype(np.float32)
    qs = slice(core * QL, (core + 1) * QL)
    out = {}
    # g tiles, pair-ordered: pair p = tiles (t, t+4), t = 8*(p//4) + p%4
    gt = g[:, qs]                                        # [B,QL,N,S,S,DG]
    g_t = np.zeros((T, DG + 1, KEY), np.float32)
    g_t[:, 0:DG, :] = gt.transpose(0, 1, 3, 5, 2, 4).reshape(T, DG, KEY)
    g_t[:, DG, :] = 1.0
    p_arr = np.arange(64)
    tA = 8 * (p_arr // 4) + (p_arr % 4)
    g_t2 = np.concatenate([g_t[tA], g_t[tA + 4]], axis=2)  # [64, 9, 1024]
    out["g_t2"] = np.ascontiguousarray(g_t2.astype(BF16))
    # Upack + residual/mask smalls
    upack = np.zeros((128, 128 * NPAIR), np.float64)
    small = np.zeros((128, 8), np.float32)
    cfq = cf[:, qs]                                      # [B,QL,S,C]
    for t in range(T):
        b, r = divmod(t, QL * S)
        ql, sq = divmod(r, S)
        P, u = divmod(t, 32)
        cg, s = u % 4, u // 4
        for c in range(C):
            fq = cfq[b, ql, sq, c]
            u_vec = G[b, c] @ _lag(np.array([fq]), nodes[b, c], cw)[0]
            row = 32 * cg + 4 * s + c
            upack[32 * c:32 * c + 32, 128 * P + row] = u_vec
            small[row, P] = fq
            small[row, 4 + P] = mask[b, core * QL + ql, sq]
    out["upack"] = upack.astype(BF16)
    out["small128"] = small
    return out


def _build_program():
    from contextlib import ExitStack
    import concourse.bass as bass
    import concourse.tile as tile
    import concourse.mybir as mybir
    from concourse import bacc
    import bass_rust

    f32 = mybir.dt.float32
    bf16 = mybir.dt.bfloat16
    AF = mybir.ActivationFunctionType
    ALU = mybir.AluOpType

    nc = bacc.Bacc("TRN2", target_bir_lowering=False, debug=False,
                   enable_asserts=False, num_devices=NCORE)

    din = {}
    for name, shape, dt in (
        ("g_t2", [64, DG + 1, 2 * KEY], bf16),
        ("w1g", [128, 128], bf16),
        ("w2full", [128, 128], bf16),
        ("w3g", [128, 256], bf16),
        ("bias128", [128, 8], f32),
        ("upack", [128, 128 * NPAIR], bf16),
        ("psin2", [128, B * KEY], bf16),
        ("psid2", [128, B * KEY], bf16),
        ("small128", [128, 8], f32),
    ):
        din[name] = nc.dram_tensor(name, shape, dt, kind="ExternalInput").ap()
    dout = nc.dram_tensor("out128", [128, NPAIR], f32, kind="ExternalOutput").ap()

    with tile.TileContext(nc) as tc, ExitStack() as ctx:
        const = ctx.enter_context(tc.tile_pool(name="const", bufs=1))
        gp = ctx.enter_context(tc.tile_pool(name="gp", bufs=3))
        work = ctx.enter_context(tc.tile_pool(name="work", bufs=2))
        ps = ctx.enter_context(tc.tile_pool(name="ps", bufs=1, space="PSUM"))
        ep = ctx.enter_context(tc.tile_pool(name="ep", bufs=2))

        # --- constants to SBUF (upack first: the warm-up burst needs it) ---
        upack_s = const.tile([128, 128 * NPAIR], bf16, name="upack_s")
        nc.sync.dma_start(upack_s[:], din["upack"][:])
        w1g_s = const.tile([128, 128], bf16, name="w1g_s")
        nc.sync.dma_start(w1g_s[:], din["w1g"][:])
        w2full_s = const.tile([128, 128], bf16, name="w2full_s")
        nc.sync.dma_start(w2full_s[:], din["w2full"][:])
        bias128_s = const.tile([128, 8], f32, name="bias128_s")
        nc.sync.dma_start(bias128_s[:], din["bias128"][:])
        small128_s = const.tile([128, 8], f32, name="small128_s")
        nc.sync.dma_start(small128_s[:], din["small128"][:])
        # big consts DMA'd from inside the loop (after the first g tiles)
        w3g_s = const.tile([128, 256], bf16, name="w3g_s")
        psin2_s = const.tile([128, B * KEY], bf16, name="psin2_s")
        psid2_s = const.tile([128, B * KEY], bf16, name="psid2_s")
        logits_all = const.tile([128, NPAIR * KEY], f32, name="logits_all")
        pfac_s = const.tile([128, NPAIR * 2 * KEY], f32, name="pfac_s")
        out_s = const.tile([128, NPAIR], f32, name="out_s")

        # --- HAM warm-up: 15 dense FULL-ARRAY (K=128, M=128) matmuls.  The
        # clock gate only *latches* 8/8 under high array occupancy, but any
        # activity then *keeps* it warm — so one full burst up front makes
        # the whole tile-packed pipeline run at 2.4 GHz. ---
        gtb = [const.tile([128, 2 * KEY], bf16, name=f"gtb{i}") for i in range(3)]
        for i in range(3):
            nc.vector.memset(gtb[i][:], 0.0)

        scratch = ps.tile([128, KEY], f32, tag="warm", bufs=1, name="scratch")
        for _ in range(11):
            nc.tensor.matmul(scratch[:], upack_s[:, 0:128], upack_s[:, 0:KEY],
                             start=True, stop=True)

        def warm_fill(n):
            # Dependency-free full-array (K=128) short matmuls: the PE runs
            # them whenever it would otherwise idle, so the HAM activity
            # monitor never sees an idle window and 2.4 GHz persists.
            for _ in range(n):
                nc.tensor.matmul(scratch[:], upack_s[:, 0:128],
                                 upack_s[:, 0:KEY], start=True, stop=True)

        # --- E_y factor maps: 8 matmuls + DVE copies to SBUF, emitted one
        # per early pipeline step (own 1-bank psum tag; fills PE gaps). ---
        def fac_stage(k):
            P, half = divmod(k, 2)
            b = P // (NPAIR // B)
            src = psin2_s if half == 0 else psid2_s
            pp = ps.tile([128, KEY], f32, tag="fac", bufs=1, name="ppF")
            nc.tensor.matmul(pp[:], upack_s[:, 128 * P:128 * (P + 1)],
                             src[:, b * KEY:(b + 1) * KEY],
                             start=True, stop=True, tile_position=(0, 0))
            nc.vector.tensor_copy(
                pfac_s[:, (2 * P + half) * KEY:(2 * P + half + 1) * KEY], pp[:])

        gts = {}
        ps1s = {}
        h1s = {}
        ps2s = {}
        h2s = {}
        ps3s = {}
        state = {"last": None}

        def dma_stage(p):
            gt = gtb[p % 3]
            nc.sync.dma_start(gt[0:DG + 1, :], din["g_t2"][p])
            gts[p] = gt

        def l1_stage(p):
            gt = gts.pop(p)
            pA = ps.tile([128, 2 * KEY], f32, tag="psL1", bufs=1, name="pA")
            nc.tensor.matmul(pA[:, 0:KEY], w1g_s[:], gt[:, 0:KEY],
                             start=True, stop=True)
            nc.tensor.matmul(pA[:, KEY:2 * KEY], w1g_s[:], gt[:, KEY:2 * KEY],
                             start=True, stop=True)
            ps1s[p] = pA

        def s1_stage(p):
            pA = ps1s.pop(p)
            h1 = work.tile([128, 2 * KEY], bf16, tag="h1", bufs=2, name="h1")
            nc.scalar.activation(h1[:], pA[:], AF.Silu, bias=0.0)
            h1s[p] = h1

        def l2_stage(p):
            h1 = h1s.pop(p)
            pB = ps.tile([128, 2 * KEY], f32, tag="psH2", bufs=1, name="pB")
            for half in range(2):
                nc.tensor.matmul(
                    pB[:, half * KEY:(half + 1) * KEY],
                    w2full_s[:],
                    h1[:, half * KEY:(half + 1) * KEY],
                    start=True, stop=True)
            ps2s[p] = pB

        def s2_stage(p):
            pB = ps2s.pop(p)
            h2 = work.tile([128, 2 * KEY], bf16, tag="h2", bufs=2, name="h2")
            nc.scalar.activation(h2[:], pB[:], AF.Silu,
                                 bias=bias128_s[:, 0:1])
            h2s[p] = h2

        def l3_stage(p):
            rho = p % 4
            chunk = p // 4
            P = chunk // 4
            s0 = 2 * (chunk % 4)
            if p % 16 == 0:
                ps3s[P] = ps.tile([128, KEY], f32, tag="psL3", bufs=2, name="ps3")
            ps3 = ps3s[P]
            h2 = h2s.pop(p)
            for half in range(2):
                s = s0 + half
                nc.tensor.matmul(
                    ps3[32 * rho:32 * rho + 32, :],
                    w3g_s[:, 32 * s:32 * s + 32],
                    h2[:, half * KEY:(half + 1) * KEY],
                    start=(s == 0), stop=(s == 7), tile_position=(0, 32 * rho))
            if p % 16 == 15:
                ps3s.pop(P)
                h = nc.scalar.activation(
                    logits_all[:, P * KEY:(P + 1) * KEY], ps3[:], AF.Silu,
                    bias=bias128_s[:, 4:5])
                state["last"] = h.ins

        # ============ phase 1: 5-deep software pipeline over 64 pairs ======
        # silu1 leads silu2 by one pair so the L2 matmuls of pair p hide
        # under silu1(p+1) instead of stalling the ACT engine.
        for step in range(64 + 4):
            if step < 64:
                dma_stage(step)
            if 3 <= step <= 66:
                l2_stage(step - 3)
            if 1 <= step <= 64:
                l1_stage(step - 1)
            if 2 <= step <= 65:
                s1_stage(step - 2)
            if 3 <= step <= 66:
                s2_stage(step - 3)
            if step >= 4:
                l3_stage(step - 4)
            if step == 1:
                nc.sync.dma_start(w3g_s[:], din["w3g"][:])
            if step == 2:
                nc.sync.dma_start(psin2_s[:], din["psin2"][:])
            if step == 3:
                nc.sync.dma_start(psid2_s[:], din["psid2"][:])
            if 10 <= step <= 17:
                fac_stage(step - 10)
            warm_fill(1)
        last_silu = state["last"]

        # ============ phase 2: exp + aggregate against factor maps =========
        for P in range(NPAIR):
            e = ep.tile([128, KEY], f32, tag="e", name="e")
            h = nc.scalar.activation(e[:], logits_all[:, P * KEY:(P + 1) * KEY],
                                     AF.Exp)
            bass_rust.add_dep_helper(h.ins, last_silu,
                                     reason="act-table phase barrier")
            import os as _os
            use_fused = _os.environ.get("K_FUSED_RED", "1") == "1"
            scrN = ep.tile([128, KEY], f32, tag="scrN", name="scrN")
            num = ep.tile([128, 1], f32, tag="num", name="num")
            scrD = ep.tile([128, KEY], f32, tag="scrD", name="scrD")
            den = ep.tile([128, 1], f32, tag="den", name="den")
            pn = pfac_s[:, P * 2 * KEY:P * 2 * KEY + KEY]
            pd = pfac_s[:, P * 2 * KEY + KEY:(P + 1) * 2 * KEY]
            if use_fused:
                nc.vector.affine_mul_reduce(scrN[:], num[:], e[:], pn, 1.0, 0.0)
                nc.vector.affine_mul_reduce(scrD[:], den[:], e[:], pd, 1.0, 0.0)
            else:
                nc.vector.tensor_mul(scrN[:], e[:], pn)
                nc.vector.tensor_reduce(num[:], scrN[:], mybir.AxisListType.X, ALU.add)
                nc.vector.tensor_mul(scrD[:], e[:], pd)
                nc.vector.tensor_reduce(den[:], scrD[:], mybir.AxisListType.X, ALU.add)
            rden = ep.tile([128, 1], f32, tag="rden", name="rden")
            nc.vector.reciprocal(rden[:], den[:])
            agg = ep.tile([128, 1], f32, tag="agg", name="agg")
            nc.vector.tensor_mul(agg[:], num[:], rden[:])
            res = ep.tile([128, 1], f32, tag="res", name="res")
            nc.vector.tensor_add(res[:], agg[:], small128_s[:, P:P + 1])
            nc.vector.tensor_mul(out_s[:, P:P + 1], res[:],
                                 small128_s[:, 4 + P:5 + P])
        nc.sync.dma_start(dout[:], out_s[:])

    nc.compile()
    return nc


def _get_program():
    global _PROG
    if _PROG is None:
        _PROG = _build_program()
    return _PROG


def _build_inmaps(inp):
    gl, aux = _pack_globals(inp)
    in_maps = []
    for core in range(NCORE):
        m = dict(gl)
        m.update(_pack_core(core, inp, aux))
        in_maps.append({k: np.ascontiguousarray(v) for k, v in m.items()})
    return in_maps


def kernel(**inputs) -> np.ndarray:
    from concourse.bass_utils import run_bass_kernel_spmd

    inp = {k: np.asarray(v) for k, v in inputs.items()}
    w_out = np.asarray(inp["w_out"], np.float32)
    in_maps = _build_inmaps(inp)
    nc = _get_program()
    res = run_bass_kernel_spmd(nc, in_maps, core_ids=list(range(NCORE)))

    cf_out = np.zeros((B, N, S, C), np.float32)
    for core in range(NCORE):
        OUT = res.results[core]["out128"]                  # [128, NPAIR]
        # row = 32*cg + 4*s + c, col = P;  t = 32*P + 4*s + cg
        arr = OUT.reshape(4, 8, C, NPAIR).transpose(3, 1, 0, 2).reshape(T, C)
        arr = arr.reshape(B, QL, S, C)
        cf_out[:, core * QL:(core + 1) * QL] = arr
    return (cf_out @ w_out.T).astype(np.float32)


# revision 29
# speedup vs baseline: 1.2493x; 1.0211x over previous
"""Trainium2 Bass kernel for nn_EquivariantMultiheadAttention.

Sharding: query-point axis (dim 1) split across 8 cores (16 points each).

Key restructure vs the straightforward version: the ky-MLP depends on just
two scalars (f_key, f_query), so exp(silu(ky3(f_q, f_k))) is approximated by
2D Chebyshev interpolation (32 nodes/dim, max rel err ~6e-6).  That yields a
rank-32-per-channel factorization E_y = U @ Psi^T evaluated as one K=128
matmul per group-pair on the tensor engine — eliminating half of all SILU
work from the activation engine (the kernel bottleneck).  mask*f_k / mask
are folded into the Psi factors so the softmax numerator/denominator come
out of two matmuls + DVE mul/reduce against exp(silu(kg3)).

Device phase 1 per tile (b, q, sq) of 512 keys: kg L1 matmul (K=9) -> SiLU
-> kg L2 as 4 concurrent 32x32 tile-positioned matmuls (output bands rotated
by tile%4 so 4 consecutive tiles pack the full PE array) -> SiLU -> L3
(zero-padded M=32 matmuls accumulating 32 tiles into one dense PSUM bank).
Phase 2 (Exp table): per group-pair (32 tiles): 2 matmuls (num/den factor
maps), exp, DVE mul/reduce/normalize, residual + query mask.
Final w_out projection happens host-side on the tiny [B,N,S,4] result.
"""
import numpy as np
import ml_dtypes

BF16 = ml_dtypes.bfloat16

B, N, S, DG, C, HID, COUT = 2, 128, 4, 8, 4, 32, 8
NCORE = 8
QL = N // NCORE          # 16 query points per core
KEY = N * S              # 512 keys
T = B * QL * S           # 128 tiles per core
M = 32                   # Chebyshev nodes per dim (= rank per channel)
NPAIR = 4                # group-pairs per core (32 tiles each)

_PROG = None


def _cheb_fit(inp):
    """Per (b, c): Chebyshev nodes + grid values G of exp(silu(ky3))."""
    cf = np.asarray(inp["coset_functions"], np.float64)
    kyW1 = np.asarray(inp["ky_W1"], np.float64)
    kyb1 = np.asarray(inp["ky_b1"], np.float64)
    kyW2 = np.asarray(inp["ky_W2"], np.float64)
    kyb2 = np.asarray(inp["ky_b2"], np.float64)
    kyW3 = np.asarray(inp["ky_W3"], np.float64)
    kyb3 = np.asarray(inp["ky_b3"], np.float64)

    def silu(x):
        return x / (1.0 + np.exp(-x))

    j = np.arange(M)
    cw = (-1.0) ** j * np.sin((2 * j + 1) * np.pi / (2 * M))
    nodes = np.zeros((B, C, M))
    G = np.zeros((B, C, M, M))
    for b in range(B):
        for c in range(C):
            f = cf[b, :, :, c].ravel()
            lo, hi = f.min(), f.max()
            x = (lo + hi) / 2 + (hi - lo) / 2 * np.cos((2 * j + 1) * np.pi / (2 * M))
            nodes[b, c] = x
            FK, FQ = np.meshgrid(x, x, indexing="ij")
            h = silu(FK[..., None] * kyW1[c, :, 0] + FQ[..., None] * kyW1[c, :, 1] + kyb1[c])
            h = silu(h @ kyW2[c].T + kyb2[c])
            G[b, c] = np.exp(silu(h @ kyW3[c].T + kyb3[c])[..., 0])  # [fk_node, fq_node]
    return nodes, G, cw


def _lag(x, nd, cw):
    """Barycentric Lagrange basis values: [len(x), M]."""
    d = x[:, None] - nd[None, :]
    ex = np.isclose(d, 0.0, atol=1e-12)
    d = np.where(ex, 1.0, d)
    L = cw[None, :] / d
    L = L / L.sum(1, keepdims=True)
    L = np.where(ex.any(1)[:, None], ex.astype(np.float64), L)
    return L


def _pack_globals(inp):
    cf = np.asarray(inp["coset_functions"], np.float64)
    mask = np.asarray(inp["mask"]).astype(np.float64)
    kgW1 = np.asarray(inp["kg_W1"], np.float32)
    kgW2 = np.asarray(inp["kg_W2"], np.float32)
    kgW3 = np.asarray(inp["kg_W3"], np.float32)
    out = {}
    # kg L1: lhsT [128, 128], rows 0-7 = g weights, row 8 = bias, rest 0
    # (K padded to 128 so the matmul streams through the full PE array).
    w1g = np.zeros((128, 128), np.float32)
    for c in range(C):
        w1g[0:DG, c * 32:(c + 1) * 32] = kgW1[c].T
    w1g[DG, :] = np.asarray(inp["kg_b1"], np.float32).reshape(128)
    out["w1g"] = w1g.astype(BF16)
    # kg L2: dense block-diagonal [128, 128] (full-array stream)
    w2full = np.zeros((128, 128), np.float32)
    for c in range(C):
        w2full[c * 32:(c + 1) * 32, c * 32:(c + 1) * 32] = kgW2[c].T
    out["w2full"] = w2full.astype(BF16)
    # kg L3: [128, 256], col 36s+c holds W3g[c] (s-slot packing)
    w3g = np.zeros((128, 256), np.float32)
    for s in range(8):
        for c in range(C):
            w3g[c * 32:(c + 1) * 32, 36 * s + c] = kgW3[c, 0, :]
    out["w3g"] = w3g.astype(BF16)
    # biases: col 0 = b2; col 4 = b3 pattern
    bias128 = np.zeros((128, 8), np.float32)
    bias128[:, 0] = np.asarray(inp["kg_b2"], np.float32).reshape(128)
    bias128[:, 4] = np.tile(np.asarray(inp["kg_b3"], np.float32).reshape(C), 32)
    out["bias128"] = bias128
    # Psi factors (num/den) per batch: [128, 2*KEY]
    nodes, G, cw = _cheb_fit(inp)
    psin = np.zeros((128, B * KEY), np.float64)
    psid = np.zeros((128, B * KEY), np.float64)
    for b in range(B):
        mk = mask[b].ravel()
        for c in range(C):
            fk = cf[b, :, :, c].ravel()
            Lk = _lag(fk, nodes[b, c], cw)            # [KEY, M]
            psin[32 * c:32 * c + 32, b * KEY:(b + 1) * KEY] = (Lk * (mk * fk)[:, None]).T
            psid[32 * c:32 * c + 32, b * KEY:(b + 1) * KEY] = (Lk * mk[:, None]).T
    out["psin2"] = psin.astype(BF16)
    out["psid2"] = psid.astype(BF16)
    return out, (nodes, G, cw)


def _pack_core(core, inp, aux):
    nodes, G, cw = aux
    g = np.asarray(inp["pairwise_g"], np.float32)
    cf = np.asarray(inp["coset_functions"], np.float64)
    mask = np.asarray(inp["mask"]).astype(np.float32)
    qs = slice(core * QL, (core + 1) * QL)
    out = {}
    # g tiles, pair-ordered: pair p = tiles (t, t+4), t = 8*(p//4) + p%4
    gt = g[:, qs]                                        # [B,QL,N,S,S,DG]
    g_t = np.zeros((T, DG + 1, KEY), np.float32)
    g_t[:, 0:DG, :] = gt.transpose(0, 1, 3, 5, 2, 4).reshape(T, DG, KEY)
    g_t[:, DG, :] = 1.0
    p_arr = np.arange(64)
    tA = 8 * (p_arr // 4) + (p_arr % 4)
    g_t2 = np.concatenate([g_t[tA], g_t[tA + 4]], axis=2)  # [64, 9, 1024]
    out["g_t2"] = np.ascontiguousarray(g_t2.astype(BF16))
    # Upack + residual/mask smalls
    upack = np.zeros((128, 128 * NPAIR), np.float64)
    small = np.zeros((128, 8), np.float32)
    cfq = cf[:, qs]                                      # [B,QL,S,C]
    for t in range(T):
        b, r = divmod(t, QL * S)
        ql, sq = divmod(r, S)
        P, u = divmod(t, 32)
        cg, s = u % 4, u // 4
        for c in range(C):
            fq = cfq[b, ql, sq, c]
            u_vec = G[b, c] @ _lag(np.array([fq]), nodes[b, c], cw)[0]
            row = 32 * cg + 4 * s + c
            upack[32 * c:32 * c + 32, 128 * P + row] = u_vec
            small[row, P] = fq
            small[row, 4 + P] = mask[b, core * QL + ql, sq]
    out["upack"] = upack.astype(BF16)
    out["small128"] = small
    return out


def _build_program():
    from contextlib import ExitStack
    import concourse.bass as bass
    import concourse.tile as tile
    import concourse.mybir as mybir
    from concourse import bacc
    import bass_rust

    f32 = mybir.dt.float32
    bf16 = mybir.dt.bfloat16
    AF = mybir.ActivationFunctionType
    ALU = mybir.AluOpType

    nc = bacc.Bacc("TRN2", target_bir_lowering=False, debug=False,
                   enable_asserts=False, num_devices=NCORE)

    din = {}
    for name, shape, dt in (
        ("g_t2", [64, DG + 1, 2 * KEY], bf16),
        ("w1g", [128, 128], bf16),
        ("w2full", [128, 128], bf16),
        ("w3g", [128, 256], bf16),
        ("bias128", [128, 8], f32),
        ("upack", [128, 128 * NPAIR], bf16),
        ("psin2", [128, B * KEY], bf16),
        ("psid2", [128, B * KEY], bf16),
        ("small128", [128, 8], f32),
    ):
        din[name] = nc.dram_tensor(name, shape, dt, kind="ExternalInput").ap()
    dout = nc.dram_tensor("out128", [128, NPAIR], f32, kind="ExternalOutput").ap()

    with tile.TileContext(nc) as tc, ExitStack() as ctx:
        const = ctx.enter_context(tc.tile_pool(name="const", bufs=1))
        gp = ctx.enter_context(tc.tile_pool(name="gp", bufs=3))
        work = ctx.enter_context(tc.tile_pool(name="work", bufs=2))
        ps = ctx.enter_context(tc.tile_pool(name="ps", bufs=1, space="PSUM"))
        ep = ctx.enter_context(tc.tile_pool(name="ep", bufs=2))

        # --- constants to SBUF (w1g + first g tiles first: shortest path
        # to the first L1 matmul; the warm-up burst uses memset zeros) ---
        w1g_s = const.tile([128, 128], bf16, name="w1g_s")
        nc.sync.dma_start(w1g_s[:], din["w1g"][:])
        upack_s = const.tile([128, 128 * NPAIR], bf16, name="upack_s")
        w2full_s = const.tile([128, 128], bf16, name="w2full_s")
        bias128_s = const.tile([128, 8], f32, name="bias128_s")
        nc.sync.dma_start(bias128_s[:], din["bias128"][:])
        small128_s = const.tile([128, 8], f32, name="small128_s")
        nc.sync.dma_start(small128_s[:], din["small128"][:])
        # big consts DMA'd from inside the loop (after the first g tiles)
        w3g_s = const.tile([128, 256], bf16, name="w3g_s")
        psin2_s = const.tile([128, B * KEY], bf16, name="psin2_s")
        psid2_s = const.tile([128, B * KEY], bf16, name="psid2_s")
        logits_all = const.tile([128, NPAIR * KEY], f32, name="logits_all")
        pfac_s = const.tile([128, NPAIR * 2 * KEY], f32, name="pfac_s")
        out_s = const.tile([128, NPAIR], f32, name="out_s")

        # --- HAM warm-up: 15 dense FULL-ARRAY (K=128, M=128) matmuls.  The
        # clock gate only *latches* 8/8 under high array occupancy, but any
        # activity then *keeps* it warm — so one full burst up front makes
        # the whole tile-packed pipeline run at 2.4 GHz. ---
        gtb = [const.tile([128, 2 * KEY], bf16, name=f"gtb{i}") for i in range(3)]
        dummy_src = const.tile([128, KEY], bf16, name="dummy_src")
        nc.vector.memset(dummy_src[:], 0.0)
        for i in range(3):
            nc.vector.memset(gtb[i][:], 0.0)

        scratch = ps.tile([128, KEY], f32, tag="warm", bufs=1, name="scratch")
        for _ in range(11):
            nc.tensor.matmul(scratch[:], dummy_src[:, 0:128], dummy_src[:],
                             start=True, stop=True)

        def warm_fill(n):
            # Dependency-free full-array (K=128) short matmuls: the PE runs
            # them whenever it would otherwise idle, so the HAM activity
            # monitor never sees an idle window and 2.4 GHz persists.
            for _ in range(n):
                nc.tensor.matmul(scratch[:], dummy_src[:, 0:128],
                                 dummy_src[:], start=True, stop=True)

        # --- E_y factor maps: 8 matmuls + DVE copies to SBUF, emitted one
        # per early pipeline step (own 1-bank psum tag; fills PE gaps). ---
        def fac_stage(k):
            P, half = divmod(k, 2)
            b = P // (NPAIR // B)
            src = psin2_s if half == 0 else psid2_s
            pp = ps.tile([128, KEY], f32, tag="fac", bufs=1, name="ppF")
            nc.tensor.matmul(pp[:], upack_s[:, 128 * P:128 * (P + 1)],
                             src[:, b * KEY:(b + 1) * KEY],
                             start=True, stop=True, tile_position=(0, 0))
            nc.vector.tensor_copy(
                pfac_s[:, (2 * P + half) * KEY:(2 * P + half + 1) * KEY], pp[:])

        gts = {}
        ps1s = {}
        h1s = {}
        ps2s = {}
        h2s = {}
        ps3s = {}
        state = {"last": None}

        def dma_stage(p):
            gt = gtb[p % 3]
            nc.sync.dma_start(gt[0:DG + 1, :], din["g_t2"][p])
            gts[p] = gt

        def l1_stage(p):
            gt = gts.pop(p)
            pA = ps.tile([128, 2 * KEY], f32, tag="psL1", bufs=1, name="pA")
            nc.tensor.matmul(pA[:, 0:KEY], w1g_s[:], gt[:, 0:KEY],
                             start=True, stop=True)
            nc.tensor.matmul(pA[:, KEY:2 * KEY], w1g_s[:], gt[:, KEY:2 * KEY],
                             start=True, stop=True)
            ps1s[p] = pA

        def s1_stage(p):
            pA = ps1s.pop(p)
            h1 = work.tile([128, 2 * KEY], bf16, tag="h1", bufs=2, name="h1")
            nc.scalar.activation(h1[:], pA[:], AF.Silu, bias=0.0)
            h1s[p] = h1

        def l2_stage(p):
            h1 = h1s.pop(p)
            pB = ps.tile([128, 2 * KEY], f32, tag="psH2", bufs=1, name="pB")
            for half in range(2):
                nc.tensor.matmul(
                    pB[:, half * KEY:(half + 1) * KEY],
                    w2full_s[:],
                    h1[:, half * KEY:(half + 1) * KEY],
                    start=True, stop=True)
            ps2s[p] = pB

        def s2_stage(p):
            pB = ps2s.pop(p)
            h2 = work.tile([128, 2 * KEY], bf16, tag="h2", bufs=2, name="h2")
            nc.scalar.activation(h2[:], pB[:], AF.Silu,
                                 bias=bias128_s[:, 0:1])
            h2s[p] = h2

        def l3_stage(p):
            rho = p % 4
            chunk = p // 4
            P = chunk // 4
            s0 = 2 * (chunk % 4)
            if p % 16 == 0:
                ps3s[P] = ps.tile([128, KEY], f32, tag="psL3", bufs=2, name="ps3")
            ps3 = ps3s[P]
            h2 = h2s.pop(p)
            for half in range(2):
                s = s0 + half
                nc.tensor.matmul(
                    ps3[32 * rho:32 * rho + 32, :],
                    w3g_s[:, 32 * s:32 * s + 32],
                    h2[:, half * KEY:(half + 1) * KEY],
                    start=(s == 0), stop=(s == 7), tile_position=(0, 32 * rho))
            if p % 16 == 15:
                ps3s.pop(P)
                h = nc.scalar.activation(
                    logits_all[:, P * KEY:(P + 1) * KEY], ps3[:], AF.Silu,
                    bias=bias128_s[:, 4:5])
                state["last"] = h.ins

        # ============ phase 1: 5-deep software pipeline over 64 pairs ======
        # silu1 leads silu2 by one pair so the L2 matmuls of pair p hide
        # under silu1(p+1) instead of stalling the ACT engine.
        dma_stage(0)
        dma_stage(1)
        nc.sync.dma_start(w2full_s[:], din["w2full"][:])
        nc.sync.dma_start(upack_s[:], din["upack"][:])
        for step in range(64 + 4):
            if 2 <= step < 64:
                dma_stage(step)
            if 3 <= step <= 66:
                l2_stage(step - 3)
            if 1 <= step <= 64:
                l1_stage(step - 1)
            if 2 <= step <= 65:
                s1_stage(step - 2)
            if 3 <= step <= 66:
                s2_stage(step - 3)
            if step >= 4:
                l3_stage(step - 4)
            if step == 1:
                nc.sync.dma_start(w3g_s[:], din["w3g"][:])
            if step == 2:
                nc.sync.dma_start(psin2_s[:], din["psin2"][:])
            if step == 3:
                nc.sync.dma_start(psid2_s[:], din["psid2"][:])
            if 10 <= step <= 17:
                fac_stage(step - 10)
            warm_fill(1)
        last_silu = state["last"]

        # ============ phase 2: exp + aggregate against factor maps =========
        numden = const.tile([128, 8], f32, name="numden")
        for P in range(NPAIR):
            e = ep.tile([128, KEY], f32, tag="e", name="e")
            h = nc.scalar.activation(e[:], logits_all[:, P * KEY:(P + 1) * KEY],
                                     AF.Exp)
            bass_rust.add_dep_helper(h.ins, last_silu,
                                     reason="act-table phase barrier")
            scrN = ep.tile([128, KEY], f32, tag="scrN", name="scrN")
            scrD = ep.tile([128, KEY], f32, tag="scrD", name="scrD")
            pn = pfac_s[:, P * 2 * KEY:P * 2 * KEY + KEY]
            pd = pfac_s[:, P * 2 * KEY + KEY:(P + 1) * 2 * KEY]
            nc.vector.affine_mul_reduce(scrN[:], numden[:, P:P + 1], e[:], pn,
                                        1.0, 0.0)
            nc.vector.affine_mul_reduce(scrD[:], numden[:, 4 + P:5 + P], e[:],
                                        pd, 1.0, 0.0)
        rden = ep.tile([128, 4], f32, tag="rden", name="rden")
        nc.vector.reciprocal(rden[:], numden[:, 4:8])
        agg = ep.tile([128, 4], f32, tag="agg", name="agg")
        nc.vector.tensor_mul(agg[:], numden[:, 0:4], rden[:])
        res = ep.tile([128, 4], f32, tag="res", name="res")
        nc.vector.tensor_add(res[:], agg[:], small128_s[:, 0:4])
        nc.vector.tensor_mul(out_s[:], res[:], small128_s[:, 4:8])
        nc.sync.dma_start(dout[:], out_s[:])

    nc.compile()
    return nc


def _get_program():
    global _PROG
    if _PROG is None:
        _PROG = _build_program()
    return _PROG


def _build_inmaps(inp):
    gl, aux = _pack_globals(inp)
    in_maps = []
    for core in range(NCORE):
        m = dict(gl)
        m.update(_pack_core(core, inp, aux))
        in_maps.append({k: np.ascontiguousarray(v) for k, v in m.items()})
    return in_maps


def kernel(**inputs) -> np.ndarray:
    from concourse.bass_utils import run_bass_kernel_spmd

    inp = {k: np.asarray(v) for k, v in inputs.items()}
    w_out = np.asarray(inp["w_out"], np.float32)
    in_maps = _build_inmaps(inp)
    nc = _get_program()
    res = run_bass_kernel_spmd(nc, in_maps, core_ids=list(range(NCORE)))

    cf_out = np.zeros((B, N, S, C), np.float32)
    for core in range(NCORE):
        OUT = res.results[core]["out128"]                  # [128, NPAIR]
        # row = 32*cg + 4*s + c, col = P;  t = 32*P + 4*s + cg
        arr = OUT.reshape(4, 8, C, NPAIR).transpose(3, 1, 0, 2).reshape(T, C)
        arr = arr.reshape(B, QL, S, C)
        cf_out[:, core * QL:(core + 1) * QL] = arr
    return (cf_out @ w_out.T).astype(np.float32)
